# revision 20
# baseline (speedup 1.0000x reference)
"""BlockDiagonalGRU Trainium2 kernel — transposed-world, block-sharded.

Math (per batch row b, per block n of 8, BLK=256):
  gates[b, n, :] = x[b, n*256:(n+1)*256] @ w_ih[n].T + h[b, ...] @ w_hh[n].T + b
  r = sigmoid(gates[..., 0:256]); u = sigmoid(gates[..., 256:512])
  c = tanh(r * gates[..., 512:768])
  h_new = (1-u)*h_blk + u*c

Sharding: one block per core (blocks are independent). The host transposes
x and h once ([B, D] -> [D, B]) so core c receives the contiguous row slab
xT[c*256:(c+1)*256] — everything on device runs in [hidden, batch] layout:

  gatesT[g, b] = sum_f W[g, f] * xT[f, b]
  => matmul(out=psum[g_chunk 128, batch 512], lhsT=wT[f 128, g 128],
            rhs=xT[f 128, batch 512])

so the weights are the PE-stationary operand and the activations stream;
no on-device transposes or PSUM-evac copies at all. Per super-tile of 512
batch cols: 24 accumulating matmuls (6 gate chunks x {x,h} x 2 k-chunks),
sigmoid r/u and tanh on ACT, candidate multiply + blend on DVE (all-bf16
operands), bf16 store (host upcasts). fp32 activations are cast-loaded to
bf16 by SWDGE; the first two super-tiles and the weights are host-pre-cast
bf16 fetched via HWDGE in first-use order so the startup burst fits the
~358GB/s per-core HBM share and the first real matmul starts ~10.5us.
"""

import numpy as np
import ml_dtypes

NUM_BLOCKS = 8
BLK = 256
D = 2048
B = 16384
N_CORES = 8
P = 128
G3 = 3 * BLK      # 768 gates per block
NG = G3 // P      # 6 gate chunks of 128
KC = BLK // P     # 2 feat chunks of 128 per source
ROWS = BLK // P   # 2 hidden row-chunks per block
NB = 512          # batch columns per super-tile (one PSUM bank of fp32)
NBT = B // NB     # 32 super-tiles per core

_nc_cache = {}


def _build(has_bias, reps=1):
    import concourse.mybir as mybir
    import concourse.tile as tile
    from concourse import bacc

    f32 = mybir.dt.float32
    bf16 = mybir.dt.bfloat16
    Sig = mybir.ActivationFunctionType.Sigmoid
    Tanh = mybir.ActivationFunctionType.Tanh
    Alu = mybir.AluOpType

    nc = bacc.Bacc(None, target_bir_lowering=False)

    x_d = nc.dram_tensor("xt", [BLK, B], f32, kind="ExternalInput")
    h_d = nc.dram_tensor("ht", [BLK, B], f32, kind="ExternalInput")
    wt_d = nc.dram_tensor("wt", [P, 2 * NG * KC * P], bf16, kind="ExternalInput")
    # first two super-tiles of x/h pre-cast to bf16 on the host: fetched via
    # HWDGE so the startup path skips the slow SWDGE descriptor generation
    xh_d = nc.dram_tensor("xh01", [P, 2 * 2 * KC * NB], bf16, kind="ExternalInput")
    if has_bias:
        bias_d = nc.dram_tensor("bias", [P, NG], f32, kind="ExternalInput")
    out_d = nc.dram_tensor("out", [BLK, B], bf16, kind="ExternalOutput")
    warm_d = nc.dram_tensor("warm_scratch", [P, P], bf16)

    with tile.TileContext(nc) as tc:
        with (
            tc.tile_pool(name="const", bufs=1) as cpool,
            tc.tile_pool(name="io", bufs=3) as io,
            tc.tile_pool(name="work", bufs=3) as work,
            tc.tile_pool(name="psr", bufs=1, space="PSUM") as psr_pool,
            tc.tile_pool(name="psu", bufs=1, space="PSUM") as psu_pool,
            tc.tile_pool(name="psc", bufs=2, space="PSUM") as psc_pool,
        ):
            warmsrc = cpool.tile([P, P], bf16)
            nc.vector.memset(warmsrc[:], 1.0)
            # weights: [128 feat_lo, s(2), g(6), kc(2), 128 gate]
            wt = cpool.tile([P, 2, NG, KC, P], bf16)
            if has_bias:
                bias_sb = cpool.tile([P, NG], f32)

            def load_tile(bt):
                c0 = bt * NB
                xt = io.tile([P, KC, NB], bf16, tag="xt", name="xt")
                ht = io.tile([P, KC, NB], bf16, tag="ht", name="ht")
                for kc in range(KC):
                    nc.gpsimd.dma_start(xt[:, kc, :], x_d[kc * P : (kc + 1) * P, c0 : c0 + NB])
                    nc.gpsimd.dma_start(ht[:, kc, :], h_d[kc * P : (kc + 1) * P, c0 : c0 + NB])
                return xt, ht

            def mm_pair(ps, g0, xt, ht):
                # accumulate gate chunks g0, g0+1 into ps [P, 2, NB]
                for j in range(2):
                    g = g0 + j
                    for s, src in ((0, xt), (1, ht)):
                        for kc in range(KC):
                            nc.tensor.matmul(
                                ps[:, j, :],
                                wt[:, s, g, kc, :],
                                src[:, kc, :],
                                start=(s == 0 and kc == 0),
                                stop=(s == 1 and kc == KC - 1),
                            )

            def drain(bt, ht, ps_r, ps_u, ps_c):
                c0 = bt * NB
                r_sb = work.tile([P, ROWS, NB], bf16, tag="r", name="r")
                u_sb = work.tile([P, ROWS, NB], bf16, tag="u", name="u")
                if has_bias:
                    for j in range(2):
                        nc.scalar.activation(r_sb[:, j, :], ps_r[:, j, :], Sig, bias=bias_sb[:, j : j + 1])
                        nc.scalar.activation(u_sb[:, j, :], ps_u[:, j, :], Sig, bias=bias_sb[:, 2 + j : 3 + j])
                else:
                    nc.scalar.activation(r_sb[:], ps_r[:], Sig)
                    nc.scalar.activation(u_sb[:], ps_u[:], Sig)
                rc = work.tile([P, ROWS, NB], bf16, tag="rc", name="rc")
                if has_bias:
                    for j in range(2):
                        nc.vector.scalar_tensor_tensor(
                            rc[:, j, :], ps_c[:, j, :], bias_sb[:, 4 + j : 5 + j], r_sb[:, j, :],
                            op0=Alu.add, op1=Alu.mult,
                        )
                else:
                    nc.vector.tensor_mul(rc[:], r_sb[:], ps_c[:])
                c_sb = work.tile([P, ROWS, NB], bf16, tag="c", name="c")
                nc.scalar.activation(c_sb[:], rc[:], Tanh)
                # h_new = h + u * (c - h), all bf16
                d_sb = work.tile([P, ROWS, NB], bf16, tag="d", name="d")
                nc.vector.tensor_sub(d_sb[:], c_sb[:], ht[:])
                e_sb = work.tile([P, ROWS, NB], bf16, tag="e", name="e")
                nc.vector.tensor_mul(e_sb[:], u_sb[:], d_sb[:])
                o_sb = work.tile([P, ROWS, NB], bf16, tag="o", name="o")
                nc.vector.tensor_add(o_sb[:], ht[:], e_sb[:])
                for kc in range(ROWS):
                    nc.sync.dma_start(out_d[kc * P : (kc + 1) * P, c0 : c0 + NB], o_sb[:, kc, :])

            def drain_tail(bt, ht, ps_r, ps_u, ps_c):
                # last super-tile: per-row-chunk drain halves the serial
                # ACT->DVE->store chain after the final matmul
                c0 = bt * NB
                for j in range(ROWS):
                    r_sb = work.tile([P, NB], bf16, tag="rj", name="rj", bufs=2)
                    u_sb = work.tile([P, NB], bf16, tag="uj", name="uj", bufs=2)
                    if has_bias:
                        nc.scalar.activation(r_sb[:], ps_r[:, j, :], Sig, bias=bias_sb[:, j : j + 1])
                        nc.scalar.activation(u_sb[:], ps_u[:, j, :], Sig, bias=bias_sb[:, 2 + j : 3 + j])
                    else:
                        nc.scalar.activation(r_sb[:], ps_r[:, j, :], Sig)
                        nc.scalar.activation(u_sb[:], ps_u[:, j, :], Sig)
                    rc = work.tile([P, NB], bf16, tag="rcj", name="rcj", bufs=2)
                    if has_bias:
                        nc.vector.scalar_tensor_tensor(
                            rc[:], ps_c[:, j, :], bias_sb[:, 4 + j : 5 + j], r_sb[:],
                            op0=Alu.add, op1=Alu.mult,
                        )
                    else:
                        nc.vector.tensor_mul(rc[:], r_sb[:], ps_c[:, j, :])
                    c_sb = work.tile([P, NB], bf16, tag="cj", name="cj", bufs=2)
                    nc.scalar.activation(c_sb[:], rc[:], Tanh)
                    d_sb = work.tile([P, NB], bf16, tag="dj", name="dj", bufs=2)
                    nc.vector.tensor_sub(d_sb[:], c_sb[:], ht[:, j, :])
                    e_sb = work.tile([P, NB], bf16, tag="ej", name="ej", bufs=2)
                    nc.vector.tensor_mul(e_sb[:], u_sb[:], d_sb[:])
                    o_sb = work.tile([P, NB], bf16, tag="oj", name="oj", bufs=2)
                    nc.vector.tensor_add(o_sb[:], ht[:, j, :], e_sb[:])
                    eng = nc.sync if j == 0 else nc.scalar
                    eng.dma_start(out_d[j * P : (j + 1) * P, c0 : c0 + NB], o_sb[:])

            def warmup():
                # dummy matmuls while the startup DMAs are in flight: keeps
                # the PE HAM activity monitor busy so real matmuls run at
                # full clock; sized to end about when the first data lands
                ps = psr_pool.tile([P, ROWS, NB], f32, tag="psr", name="psr_warm")
                NWU = 26
                for i in range(NWU):
                    nc.tensor.matmul(
                        ps[:, 0, 0:P],
                        warmsrc[:],
                        warmsrc[:],
                        start=(i == 0),
                        stop=(i == NWU - 1),
                    )
                sc = work.tile([P, P], bf16, tag="warm_sb", name="warm_sb", bufs=1)
                nc.vector.tensor_copy(sc[:], ps[:, 0, 0:P])
                nc.scalar.dma_start(warm_d[:, :], sc[:])
                return sc

            def body(_iv=None):
                # startup DMAs in first-use order (the burst is bandwidth
                # bound): r weights, x/h tile 0, u weights, tile 1, c weights
                CH = KC * NB

                def load_wt(g0, g1):
                    for s in range(2):
                        nc.scalar.dma_start(
                            wt[:, s, g0:g1, :, :],
                            wt_d[:, s * (NG * KC * P) + g0 * KC * P : s * (NG * KC * P) + g1 * KC * P],
                        )

                def load_first(t):
                    xt = io.tile([P, KC, NB], bf16, tag="xt", name="xt")
                    ht = io.tile([P, KC, NB], bf16, tag="ht", name="ht")
                    nc.sync.dma_start(xt[:], xh_d[:, (t * 2) * CH : (t * 2 + 1) * CH])
                    nc.sync.dma_start(ht[:], xh_d[:, (t * 2 + 1) * CH : (t * 2 + 2) * CH])
                    return xt, ht

                load_wt(0, 2)
                t0 = load_first(0)
                load_wt(2, 4)
                t1 = load_first(1)
                load_wt(4, 6)
                if has_bias:
                    nc.scalar.dma_start(bias_sb[:], bias_d[:, :])
                tiles = {0: t0, 1: t1}
                sc = warmup()
                # tile 2: tiny WAW writes that depend on the warmup drain
                # defer its SWDGE transfers past the startup HWDGE burst
                # (SWDGE descriptor gens otherwise fire immediately and
                # steal HBM bandwidth from the first-use loads)
                xt2 = io.tile([P, KC, NB], bf16, tag="xt", name="xt")
                ht2 = io.tile([P, KC, NB], bf16, tag="ht", name="ht")
                nc.vector.tensor_copy(xt2[:, 0, 0:P], sc[:])
                nc.vector.tensor_copy(ht2[:, 0, 0:P], sc[:])
                for kc in range(KC):
                    nc.gpsimd.dma_start(xt2[:, kc, :], x_d[kc * P : (kc + 1) * P, 2 * NB : 2 * NB + NB])
                    nc.gpsimd.dma_start(ht2[:, kc, :], h_d[kc * P : (kc + 1) * P, 2 * NB : 2 * NB + NB])
                tiles[2] = (xt2, ht2)
                for bt in range(NBT):
                    xt, ht = tiles.pop(bt)
                    ps_r = psr_pool.tile([P, ROWS, NB], f32, tag="psr", name="psr")
                    ps_u = psu_pool.tile([P, ROWS, NB], f32, tag="psu", name="psu")
                    ps_c = psc_pool.tile([P, ROWS, NB], f32, tag="psc", name="psc")
                    mm_pair(ps_r, 0, xt, ht)
                    if bt == 0:
                        tiles[3] = load_tile(3)
                    elif bt + 3 < NBT:
                        tiles[bt + 3] = load_tile(bt + 3)
                    mm_pair(ps_u, 2, xt, ht)
                    mm_pair(ps_c, 4, xt, ht)
                    if bt == NBT - 1:
                        drain_tail(bt, ht, ps_r, ps_u, ps_c)
                    else:
                        drain(bt, ht, ps_r, ps_u, ps_c)

            if reps == 1:
                body()
            else:
                with tc.For_i(0, reps, 1) as iv:
                    body(iv)

    nc.compile()
    return nc


def _get_nc(has_bias, reps=1):
    key = (has_bias, reps)
    if key not in _nc_cache:
        _nc_cache[key] = _build(has_bias, reps)
    return _nc_cache[key]


def _prep_weights(w_ih, w_hh):
    # wt[c][p, s, g, kc, j] = W_s[c][g*128 + j, kc*128 + p]
    w = np.stack([w_ih, w_hh], axis=1)          # [c, s, 768, 256]
    w = w.reshape(NUM_BLOCKS, 2, NG, P, KC, P)  # [c, s, g, j, kc, p]
    w = w.transpose(0, 5, 1, 2, 4, 3)           # [c, p, s, g, kc, j]
    return np.ascontiguousarray(
        w.reshape(NUM_BLOCKS, P, -1).astype(ml_dtypes.bfloat16)
    )


def _make_in_maps(x, h, w_ih, w_hh, b_ih, b_hh):
    x = np.asarray(x, dtype=np.float32)
    h = np.asarray(h, dtype=np.float32)
    w_ih = np.asarray(w_ih, dtype=np.float32)
    w_hh = np.asarray(w_hh, dtype=np.float32)
    bsum = np.asarray(b_ih, dtype=np.float32) + np.asarray(b_hh, dtype=np.float32)
    has_bias = bool(np.any(bsum))

    xT = np.ascontiguousarray(x.T)  # [D, B]
    hT = np.ascontiguousarray(h.T)
    wt = _prep_weights(w_ih, w_hh)

    in_maps = []
    for c in range(N_CORES):
        xTc = xT[c * BLK : (c + 1) * BLK]
        hTc = hT[c * BLK : (c + 1) * BLK]
        # first two super-tiles pre-cast to bf16: [p, t, s, kc, col]
        xh01 = (
            np.stack([xTc[:, : 2 * NB], hTc[:, : 2 * NB]], axis=0)
            .reshape(2, KC, P, 2, NB)
            .transpose(2, 3, 0, 1, 4)
            .reshape(P, -1)
            .astype(ml_dtypes.bfloat16)
        )
        m = {
            "xt": np.ascontiguousarray(xTc),
            "ht": np.ascontiguousarray(hTc),
            "wt": wt[c],
            "xh01": np.ascontiguousarray(xh01),
        }
        if has_bias:
            # bias_sb[p, g] = bsum[c, g*128 + p]
            m["bias"] = np.ascontiguousarray(
                bsum[c].reshape(NG, P).T.astype(np.float32)
            )
        in_maps.append(m)
    return in_maps, has_bias


def _gather(results):
    outT = np.concatenate(
        [np.asarray(results[c]["out"]) for c in range(N_CORES)], axis=0
    )  # [D, B] bf16
    return np.ascontiguousarray(outT.T.astype(np.float32))


def kernel(x, h, w_ih, w_hh, b_ih, b_hh, _reps=1, _nc=None):
    from concourse.bass_utils import run_bass_kernel_spmd

    in_maps, has_bias = _make_in_maps(x, h, w_ih, w_hh, b_ih, b_hh)
    nc = _nc if _nc is not None else _get_nc(has_bias, _reps)
    res = run_bass_kernel_spmd(nc, in_maps, core_ids=list(range(N_CORES)))
    return _gather(res.results)
